# revision 30
# baseline (speedup 1.0000x reference)
"""MoE layer (dense routing, E=8 experts, top_k=E) Trainium2 Bass kernel.

Problem (hardcoded): x [4, 2048, 1024] f32, Wg [1024, 8], bg [8],
W1 [8, 1024, 256], b1 [8, 256], W2 [8, 256, 1024], b2 [8, 1024].

reference:
    logits = x @ Wg + bg ; probs = softmax(logits)
    sorted_probs = sort(probs, descending)          # top_k with k=E
    h_e = gelu(x @ W1[e] + b1[e])                   # all experts, all tokens
    out = sum_e (h_e @ W2[e] + b2[e]) * sorted_probs[..., e, None]

Sharding: data-parallel over the 8192 tokens -> 1024 tokens/core, 8 cores,
weights replicated, no collectives.

Per-core layout: activations are feature-major ([D, tok] / [H, tok] /
[DOUT, tok]); the host pre-transposes x and un-transposes the output.
All big matmuls are bf16 (fp32 PSUM accumulation). Gating softmax + the
descending sort (iterative max-extraction) run token-major on DVE in a
rank-major [128, E, TG] layout; one PE transpose of the flat [128, 64]
view yields [e*TG+tg, tok] partitions, one DVE copy + one DMA lays that
out as bf16 [E, TOK] in DRAM, and per-expert stride-0-partition DMAs
broadcast each row across 128 partitions. gelu(bias) runs on Act over a
paired 2-bank PSUM tile; the per-token weighting mul runs bf16 on DVE.
The second matmul accumulates all 8 experts (and the b2 @ w term) into
one PSUM tile per output chunk.

Loop mode emits 4 bodies per For_i iteration (staggered_reset, h/wT
double-buffered by body parity, a no_sync scheduling fence before each
body's second GEMM) so consecutive bodies pipeline across engines.
test.py uses the marginal time between loop counts to measure HW exec
time through the fixed ~40ms axon dispatch overhead, timing the two
points in alternating pairs so slow device-clock drift cancels.
"""

import sys

if "/opt/trn_rl_repo" not in sys.path:
    sys.path.insert(0, "/opt/trn_rl_repo")

import numpy as np
import ml_dtypes

import concourse.bass as bass
import concourse.mybir as mybir
import concourse.tile as tile
from concourse import bacc
from concourse.masks import make_identity

B, S, D, DOUT = 4, 2048, 1024, 1024
E, H = 8, 256
NCORES = 8
TOK = (B * S) // NCORES  # 1024 tokens per core
DC = D // 128            # 8 contraction chunks over D
HC = H // 128            # 2 chunks over H
OC = DOUT // 128         # 8 chunks over DOUT
TN = TOK // 512          # 2 moving-dim chunks of 512 tokens
TG = TOK // 128          # 8 token groups of 128 (partition tiles)

BF16 = mybir.dt.bfloat16
F32 = mybir.dt.float32

_CACHE = {}
GELU_FUNC = mybir.ActivationFunctionType.Gelu  # sim override hook


def _bcast_inner(ap2d, n):
    """[P, G] (or [P, G, 1]) AP -> [P, G, n] AP with stride-0 innermost."""
    a = [list(d) for d in ap2d.ap]
    if len(a) == 3:
        assert a[2][1] == 1
        a = a[:2]
    return bass.AP(tensor=ap2d.tensor, offset=ap2d.offset, ap=a + [[0, n]])


def build_nc(reps=1, loop_n=None, skip_bg=False, skip_b2=False):
    nc = bacc.Bacc("TRN2", target_bir_lowering=False, debug=False,
                   num_devices=NCORES)

    xT_d = nc.dram_tensor("xT", [D, TOK], BF16, kind="ExternalInput")
    Wg_d = nc.dram_tensor("Wg", [D, E], BF16, kind="ExternalInput")
    bg_d = nc.dram_tensor("bg", [1, E], F32, kind="ExternalInput")
    W1_d = nc.dram_tensor("W1", [E, D, H], BF16, kind="ExternalInput")
    b1_d = nc.dram_tensor("b1", [E, HC, 128, 1], F32, kind="ExternalInput")
    W2_d = nc.dram_tensor("W2", [E, H, DOUT], BF16, kind="ExternalInput")
    b2_d = nc.dram_tensor("b2", [E, DOUT], BF16, kind="ExternalInput")
    outT_d = nc.dram_tensor("outT", [DOUT, TOK], BF16, kind="ExternalOutput")
    wTd2 = [nc.dram_tensor(f"wT_scratch{s}", [E, TOK], BF16) for s in range(2)]

    with tile.TileContext(nc) as tc:
        with (
            tc.tile_pool(name="const", bufs=1) as const,
            tc.tile_pool(name="work", bufs=4) as work,
            tc.tile_pool(name="ps_small", bufs=1, space="PSUM") as ps_small,
            tc.tile_pool(name="ps_h", bufs=2, space="PSUM") as ps_h,
            tc.tile_pool(name="ps_out", bufs=3, space="PSUM") as ps_out,
        ):
            # ---- resident inputs ------------------------------------------
            xT_sb = []
            for dc in range(DC):
                t = const.tile([128, TOK], BF16, name=f"xT{dc}")
                nc.sync.dma_start(t, xT_d[dc * 128:(dc + 1) * 128, :])
                xT_sb.append(t)
            Wg_sb = []
            for dc in range(DC):
                t = const.tile([128, E], BF16, name=f"Wg{dc}")
                nc.sync.dma_start(t, Wg_d[dc * 128:(dc + 1) * 128, :])
                Wg_sb.append(t)
            bg_sb = const.tile([1, E], F32, name="bg")
            nc.sync.dma_start(bg_sb, bg_d[:, :])
            ones_sb = const.tile([1, 128], F32, name="ones")
            nc.vector.memset(ones_sb, 1.0)
            ident = const.tile([128, 128], F32, name="ident")
            make_identity(nc, ident)
            ident_bf = const.tile([128, 128], BF16, name="ident_bf")
            nc.vector.tensor_copy(ident_bf, ident)

            W1_sb = [[None] * DC for _ in range(E)]
            b1_sb = [[None] * HC for _ in range(E)]
            for e in range(E):
                for dc in range(DC):
                    t = const.tile([128, H], BF16, name=f"W1_{e}_{dc}")
                    nc.sync.dma_start(t, W1_d[e, dc * 128:(dc + 1) * 128, :])
                    W1_sb[e][dc] = t
                for hc in range(HC):
                    t = const.tile([128, 1], F32, name=f"b1_{e}_{hc}")
                    nc.sync.dma_start(t, b1_d[e, hc, :, :])
                    b1_sb[e][hc] = t
            W2_sb = [[None] * HC for _ in range(E)]
            for e in range(E):
                for hc in range(HC):
                    t = const.tile([128, DOUT], BF16, name=f"W2_{e}_{hc}")
                    nc.sync.dma_start(t, W2_d[e, hc * 128:(hc + 1) * 128, :])
                    W2_sb[e][hc] = t
            b2_sb = const.tile([E, DOUT], BF16, name="b2")
            nc.sync.dma_start(b2_sb, b2_d[:, :])

            if loop_n is not None:
                n4, rem = divmod(loop_n, 4)
                if n4 > 0:
                    with tc.For_i(0, n4, 1,
                                  hint_engines=(mybir.EngineType.PE,),
                                  staggered_reset=True):
                        for s in range(4):
                            _emit_body(nc, tc, f"lp{s}_", s % 2, const, work,
                                       ps_small, ps_small, ps_h, ps_out,
                                       xT_sb, Wg_sb, bg_sb, ones_sb,
                                       wTd2[s % 2], ident_bf, W1_sb, b1_sb,
                                       W2_sb, b2_sb, outT_d, skip_bg,
                                       skip_b2)
                for r in range(rem):
                    _emit_body(nc, tc, f"lr{r}_", r % 2, const, work,
                               ps_small, ps_small, ps_h, ps_out, xT_sb,
                               Wg_sb, bg_sb, ones_sb, wTd2[r % 2], ident_bf,
                               W1_sb, b1_sb, W2_sb, b2_sb, outT_d,
                               skip_bg, skip_b2)
            else:
                for rep in range(reps):
                    _emit_body(nc, tc, f"r{rep}_", rep % 2, const, work,
                               ps_small, ps_small, ps_h, ps_out, xT_sb,
                               Wg_sb, bg_sb, ones_sb, wTd2[rep % 2], ident_bf,
                               W1_sb, b1_sb, W2_sb, b2_sb, outT_d,
                               skip_bg, skip_b2)

    nc.compile()
    return nc


def _emit_body(nc, tc, R, slot, const, work, ps_small, ps_wb, ps_h, ps_out,
               xT_sb, Wg_sb, bg_sb, ones_sb, wTd, ident,
               W1_sb, b1_sb, W2_sb, b2_sb, outT_d, skip_bg=False,
               skip_b2=False):
    # ---- gating: logits token-major [128, tg, e] --------------------------
    L_sb = const.tile([128, TG, E], F32, name=R + "L", tag="L")
    for tg in range(TG):
        psl = ps_small.tile([128, E], F32, name=f"{R}psl{tg}", tag="small")
        for dc in range(DC):
            nc.tensor.matmul(
                psl, xT_sb[dc][:, tg * 128:(tg + 1) * 128], Wg_sb[dc],
                start=(dc == 0), stop=(skip_bg and dc == DC - 1))
        if not skip_bg:
            nc.tensor.matmul(psl, ones_sb, bg_sb, start=False, stop=True)
        nc.scalar.copy(L_sb[:, tg, :], psl)

    # ---- softmax over E (innermost) ---------------------------------------
    mx = const.tile([128, TG], F32, name=R + "mx", tag="mx")
    nc.vector.reduce_max(out=mx, in_=L_sb, axis=mybir.AxisListType.X)
    P8 = const.tile([128, TG, E], F32, name=R + "P8", tag="P8")
    nc.vector.tensor_sub(P8, L_sb, _bcast_inner(mx, E))
    nc.scalar.activation(P8, P8, mybir.ActivationFunctionType.Exp)
    sm = const.tile([128, TG], F32, name=R + "sm", tag="sm")
    nc.vector.reduce_sum(out=sm, in_=P8, axis=mybir.AxisListType.X)
    rs = const.tile([128, TG], F32, name=R + "rs", tag="rs")
    nc.vector.reciprocal(rs, sm)
    nc.vector.tensor_mul(P8, P8, _bcast_inner(rs, E))

    # ---- sort descending: iterative max extraction ------------------------
    # rank-major layout [128, rank, tg]: one PE transpose of the flat [128,
    # 64] view then yields [e*TG+tg, tok] partitions, which a single DMA can
    # lay out as [E, TOK] in DRAM.
    ws = const.tile([128, E, TG], F32, name=R + "ws", tag="ws")
    eq = const.tile([128, TG, E], F32, name=R + "eq", tag="eq")
    for r in range(E):
        nc.vector.reduce_max(out=ws[:, r, :], in_=P8,
                             axis=mybir.AxisListType.X)
        if r < E - 1:
            nc.vector.tensor_tensor(
                eq, P8, _bcast_inner(ws[:, r, :], E),
                op=mybir.AluOpType.is_equal)
            # P8 += -2 * eq  (knock out the extracted max)
            nc.vector.scalar_tensor_tensor(
                P8, eq, -2.0, P8,
                op0=mybir.AluOpType.mult, op1=mybir.AluOpType.add)

    # ---- experts ----------------------------------------------------------
    # Emission order keeps PE streaming: expert 0's first matmuls are emitted
    # before the sorted-weight transposes, so the PE fills the DVE sort
    # latency with useful work. Sorted weights go to bf16 hi+lo pairs so the
    # broadcast / b2 matmuls run at bf16 rate (w = hi + lo, exact to 2^-18).
    h_sb = [[const.tile([128, TOK], BF16, name=f"{R}h_{e}_{hc}",
                        tag=f"h_{e}_{hc}s{slot}")
             for hc in range(HC)] for e in range(E)]
    # [e*TG+tg, 128tok] — partition-order matches DRAM [E, TOK] exactly
    wT_sb = const.tile([E * TG, 128], BF16, name=R + "wT", tag=f"wTs{slot}")

    def emit_ph_pair(e, hc):
        # both token halves accumulate into one 2-bank PSUM tile; single gelu
        ph = ps_h.tile([128, TN, 512], F32, name=f"{R}ph{e}_{hc}", tag="ph")
        for dc in range(DC):
            w_ap = W1_sb[e][dc][:, hc * 128:(hc + 1) * 128]
            for t in range(TN):
                mm = nc.tensor.matmul(ph[:, t, :], w_ap,
                                      xT_sb[dc][:, t * 512:(t + 1) * 512],
                                      start=(dc == 0), stop=(dc == DC - 1))
                if t > 0:
                    mm.ins.ldweights = False
        gt = work.tile([128, TN, 512], BF16, name=f"{R}gt{e}_{hc}",
                       tag="gt", bufs=8)
        nc.scalar.activation(gt, ph, GELU_FUNC, bias=b1_sb[e][hc])
        return gt

    def emit_wb(e, tn):
        # replicate bf16 w row e across 128 partitions via DMA broadcast
        # from the DRAM bounce (stride-0 partition reads need a DRAM source)
        row = wTd[e:e + 1, tn * 512:(tn + 1) * 512]
        bcast = bass.AP(tensor=row.tensor, offset=row.offset,
                        ap=[[0, 128]] + [list(d) for d in row.ap[1:]])
        wb = work.tile([128, 512], BF16, name=f"{R}wb{e}_{tn}", tag="wbs")
        nc.sync.dma_start(wb, bcast)
        return wb

    # experts 0-2: matmuls first (PE cover while DVE runs softmax + sort)
    NPRE = 3
    gt_pre = {e: [emit_ph_pair(e, hc) for hc in range(HC)]
              for e in range(NPRE)}

    # sorted weights -> bf16 -> one PE transpose of the flat [128, 64] view
    # -> [64, 128] psum -> one DVE copy -> one DMA to the DRAM bounce
    ws_bf = const.tile([128, E * TG], BF16, name=R + "wsbf", tag="wsbf")
    nc.vector.tensor_copy(ws_bf, ws)
    pst = ps_out.tile([E * TG, 128], BF16, name=R + "pst", tag="po")
    nc.tensor.transpose(pst, ws_bf, ident)
    nc.vector.tensor_copy(wT_sb, pst)
    wTd_flat = bass.AP(tensor=wTd[:, :].tensor, offset=0,
                       ap=[[128, E * TG], [1, 128]])
    nc.sync.dma_start(wTd_flat, wT_sb)

    for e in range(NPRE):
        wbs = [emit_wb(e, tn) for tn in range(TN)]
        for hc in range(HC):
            for tn in range(TN):
                tsl = slice(tn * 512, (tn + 1) * 512)
                nc.vector.tensor_mul(h_sb[e][hc][:, tsl],
                                     gt_pre[e][hc][:, tn, :], wbs[tn])

    for e in range(NPRE, E):
        wbs = [emit_wb(e, tn) for tn in range(TN)]
        for hc in range(HC):
            gt = emit_ph_pair(e, hc)
            for tn in range(TN):
                tsl = slice(tn * 512, (tn + 1) * 512)
                nc.vector.tensor_mul(h_sb[e][hc][:, tsl], gt[:, tn, :],
                                     wbs[tn])

    # Scheduler fence: keep the second GEMM after all first-GEMM work so the
    # PE never reaches an h-consuming matmul before its producers have cover.
    tc.no_sync_barrier()

    for oc in range(OC):
        pos = [ps_out.tile([128, 512], F32, name=f"{R}po{oc}_{t}", tag="po")
               for t in range(TN)]
        if not skip_b2:
            # bf16 b2 x bf16 wT (rare path: only when b2 is nonzero)
            for t in range(TN):
                mm = nc.tensor.matmul(pos[t], b2_sb[:, oc * 128:(oc + 1) * 128],
                                      wT_sb[:, t * 512:(t + 1) * 512],
                                      start=True, stop=False)
                if t > 0:
                    mm.ins.ldweights = False
        for e in range(E):
            for hc in range(HC):
                w_ap = W2_sb[e][hc][:, oc * 128:(oc + 1) * 128]
                for t in range(TN):
                    mm = nc.tensor.matmul(
                        pos[t], w_ap, h_sb[e][hc][:, t * 512:(t + 1) * 512],
                        start=(skip_b2 and e == 0 and hc == 0),
                        stop=(e == E - 1 and hc == HC - 1))
                    if t > 0:
                        mm.ins.ldweights = False
        for t in range(TN):
            ot = work.tile([128, 512], BF16, name=f"{R}ot{oc}_{t}", tag="ot")
            if t == 0:
                nc.vector.tensor_copy(ot, pos[t])
            else:
                nc.scalar.copy(ot, pos[t])
            nc.sync.dma_start(
                outT_d[oc * 128:(oc + 1) * 128, t * 512:(t + 1) * 512], ot)


def _prep_in_maps(x, Wg, bg, W1, b1, W2, b2):
    x = np.asarray(x, dtype=np.float32).reshape(B * S, D)
    Wg_bf = np.asarray(Wg, dtype=np.float32).astype(ml_dtypes.bfloat16)
    bg_f = np.asarray(bg, dtype=np.float32).reshape(1, E)
    W1_bf = np.asarray(W1, dtype=np.float32).astype(ml_dtypes.bfloat16)
    b1_f = np.ascontiguousarray(
        np.asarray(b1, dtype=np.float32).reshape(E, HC, 128, 1))
    W2_bf = np.asarray(W2, dtype=np.float32).astype(ml_dtypes.bfloat16)
    b2_f = np.asarray(b2, dtype=np.float32).astype(ml_dtypes.bfloat16)
    in_maps = []
    for c in range(NCORES):
        xc = x[c * TOK:(c + 1) * TOK]                      # [TOK, D]
        xT = np.ascontiguousarray(xc.T).astype(ml_dtypes.bfloat16)
        in_maps.append({
            "xT": xT, "Wg": Wg_bf, "bg": bg_f, "W1": W1_bf,
            "b1": b1_f, "W2": W2_bf, "b2": b2_f,
        })
    return in_maps


def kernel(x, Wg, bg, W1, b1, W2, b2):
    from concourse.bass_utils import run_bass_kernel_spmd

    zbg = not np.any(np.asarray(bg, dtype=np.float32))
    zb2 = not np.any(np.asarray(b2, dtype=np.float32))
    key = ("nc", zbg, zb2)
    if key not in _CACHE:
        _CACHE[key] = build_nc(skip_bg=zbg, skip_b2=zb2)
    nc = _CACHE[key]
    in_maps = _prep_in_maps(x, Wg, bg, W1, b1, W2, b2)
    res = run_bass_kernel_spmd(nc, in_maps, core_ids=list(range(NCORES)))
    out = np.empty((B * S, DOUT), dtype=np.float32)
    for c in range(NCORES):
        out[c * TOK:(c + 1) * TOK] = res.results[c]["outT"].T
    return out.reshape(B, S, DOUT)



# revision 31
# speedup vs baseline: 1.0327x; 1.0327x over previous
"""MoE layer (dense routing, E=8 experts, top_k=E) Trainium2 Bass kernel.

Problem (hardcoded): x [4, 2048, 1024] f32, Wg [1024, 8], bg [8],
W1 [8, 1024, 256], b1 [8, 256], W2 [8, 256, 1024], b2 [8, 1024].

reference:
    logits = x @ Wg + bg ; probs = softmax(logits)
    sorted_probs = sort(probs, descending)          # top_k with k=E
    h_e = gelu(x @ W1[e] + b1[e])                   # all experts, all tokens
    out = sum_e (h_e @ W2[e] + b2[e]) * sorted_probs[..., e, None]

Sharding: data-parallel over the 8192 tokens -> 1024 tokens/core, 8 cores,
weights replicated, no collectives.

Per-core layout: activations are feature-major ([D, tok] / [H, tok] /
[DOUT, tok]); the host pre-transposes x and un-transposes the output.
All big matmuls are bf16 (fp32 PSUM accumulation). Gating softmax + the
descending sort (iterative max-extraction) run token-major on DVE in a
rank-major [128, E, TG] layout; one PE transpose of the flat [128, 64]
view yields [e*TG+tg, tok] partitions, one DVE copy + one DMA lays that
out as bf16 [E, TOK] in DRAM, and per-expert stride-0-partition DMAs
broadcast each row across 128 partitions. gelu(bias) runs on Act over a
paired 2-bank PSUM tile; the per-token weighting mul runs bf16 on DVE.
The second matmul accumulates all 8 experts (and the b2 @ w term) into
one PSUM tile per output chunk.

Loop mode emits 4 bodies per For_i iteration (staggered_reset, h/wT
double-buffered by body parity, a no_sync scheduling fence before each
body's second GEMM) so consecutive bodies pipeline across engines.
test.py uses the marginal time between loop counts to measure HW exec
time through the fixed ~40ms axon dispatch overhead, timing the two
points in alternating pairs so slow device-clock drift cancels.
"""

import sys

if "/opt/trn_rl_repo" not in sys.path:
    sys.path.insert(0, "/opt/trn_rl_repo")

import numpy as np
import ml_dtypes

import concourse.bass as bass
import concourse.mybir as mybir
import concourse.tile as tile
from concourse import bacc
from concourse.masks import make_identity

B, S, D, DOUT = 4, 2048, 1024, 1024
E, H = 8, 256
NCORES = 8
TOK = (B * S) // NCORES  # 1024 tokens per core
DC = D // 128            # 8 contraction chunks over D
HC = H // 128            # 2 chunks over H
OC = DOUT // 128         # 8 chunks over DOUT
TN = TOK // 512          # 2 moving-dim chunks of 512 tokens
TG = TOK // 128          # 8 token groups of 128 (partition tiles)

BF16 = mybir.dt.bfloat16
F32 = mybir.dt.float32

_CACHE = {}
GELU_FUNC = mybir.ActivationFunctionType.Gelu  # sim override hook


def _bcast_inner(ap2d, n):
    """[P, G] (or [P, G, 1]) AP -> [P, G, n] AP with stride-0 innermost."""
    a = [list(d) for d in ap2d.ap]
    if len(a) == 3:
        assert a[2][1] == 1
        a = a[:2]
    return bass.AP(tensor=ap2d.tensor, offset=ap2d.offset, ap=a + [[0, n]])


def build_nc(reps=1, loop_n=None, skip_bg=False, skip_b2=False):
    nc = bacc.Bacc("TRN2", target_bir_lowering=False, debug=False,
                   num_devices=NCORES)

    xT_d = nc.dram_tensor("xT", [D, TOK], BF16, kind="ExternalInput")
    Wg_d = nc.dram_tensor("Wg", [D, E], BF16, kind="ExternalInput")
    bg_d = nc.dram_tensor("bg", [1, E], F32, kind="ExternalInput")
    W1_d = nc.dram_tensor("W1", [E, D, H], BF16, kind="ExternalInput")
    b1_d = nc.dram_tensor("b1", [E, HC, 128, 1], F32, kind="ExternalInput")
    W2_d = nc.dram_tensor("W2", [E, H, DOUT], BF16, kind="ExternalInput")
    b2_d = nc.dram_tensor("b2", [E, DOUT], BF16, kind="ExternalInput")
    outT_d = nc.dram_tensor("outT", [DOUT, TOK], BF16, kind="ExternalOutput")
    wTd2 = [nc.dram_tensor(f"wT_scratch{s}", [E, TOK], BF16) for s in range(2)]

    with tile.TileContext(nc) as tc:
        with (
            tc.tile_pool(name="const", bufs=1) as const,
            tc.tile_pool(name="work", bufs=4) as work,
            tc.tile_pool(name="ps_small", bufs=1, space="PSUM") as ps_small,
            tc.tile_pool(name="ps_h", bufs=2, space="PSUM") as ps_h,
            tc.tile_pool(name="ps_out", bufs=3, space="PSUM") as ps_out,
        ):
            # ---- resident inputs ------------------------------------------
            xT_sb = []
            for dc in range(DC):
                t = const.tile([128, TOK], BF16, name=f"xT{dc}")
                nc.sync.dma_start(t, xT_d[dc * 128:(dc + 1) * 128, :])
                xT_sb.append(t)
            Wg_sb = []
            for dc in range(DC):
                t = const.tile([128, E], BF16, name=f"Wg{dc}")
                nc.sync.dma_start(t, Wg_d[dc * 128:(dc + 1) * 128, :])
                Wg_sb.append(t)
            bg_sb = const.tile([1, E], F32, name="bg")
            nc.sync.dma_start(bg_sb, bg_d[:, :])
            ones_sb = const.tile([1, 128], F32, name="ones")
            nc.vector.memset(ones_sb, 1.0)
            ident = const.tile([128, 128], F32, name="ident")
            make_identity(nc, ident)
            ident_bf = const.tile([128, 128], BF16, name="ident_bf")
            nc.vector.tensor_copy(ident_bf, ident)

            W1_sb = [[None] * DC for _ in range(E)]
            b1_sb = [[None] * HC for _ in range(E)]
            for e in range(E):
                for dc in range(DC):
                    t = const.tile([128, H], BF16, name=f"W1_{e}_{dc}")
                    nc.sync.dma_start(t, W1_d[e, dc * 128:(dc + 1) * 128, :])
                    W1_sb[e][dc] = t
                for hc in range(HC):
                    t = const.tile([128, 1], F32, name=f"b1_{e}_{hc}")
                    nc.sync.dma_start(t, b1_d[e, hc, :, :])
                    b1_sb[e][hc] = t
            W2_sb = [[None] * HC for _ in range(E)]
            for e in range(E):
                for hc in range(HC):
                    t = const.tile([128, DOUT], BF16, name=f"W2_{e}_{hc}")
                    nc.sync.dma_start(t, W2_d[e, hc * 128:(hc + 1) * 128, :])
                    W2_sb[e][hc] = t
            b2_sb = const.tile([E, DOUT], BF16, name="b2")
            nc.sync.dma_start(b2_sb, b2_d[:, :])

            if loop_n is not None:
                n4, rem = divmod(loop_n, 4)
                if n4 > 0:
                    with tc.For_i(0, n4, 1,
                                  hint_engines=(mybir.EngineType.PE,),
                                  staggered_reset=True):
                        for s in range(4):
                            _emit_body(nc, tc, f"lp{s}_", s % 2, const, work,
                                       ps_small, ps_small, ps_h, ps_out,
                                       xT_sb, Wg_sb, bg_sb, ones_sb,
                                       wTd2[s % 2], ident_bf, W1_sb, b1_sb,
                                       W2_sb, b2_sb, outT_d, skip_bg,
                                       skip_b2)
                for r in range(rem):
                    _emit_body(nc, tc, f"lr{r}_", r % 2, const, work,
                               ps_small, ps_small, ps_h, ps_out, xT_sb,
                               Wg_sb, bg_sb, ones_sb, wTd2[r % 2], ident_bf,
                               W1_sb, b1_sb, W2_sb, b2_sb, outT_d,
                               skip_bg, skip_b2)
            else:
                for rep in range(reps):
                    _emit_body(nc, tc, f"r{rep}_", rep % 2, const, work,
                               ps_small, ps_small, ps_h, ps_out, xT_sb,
                               Wg_sb, bg_sb, ones_sb, wTd2[rep % 2], ident_bf,
                               W1_sb, b1_sb, W2_sb, b2_sb, outT_d,
                               skip_bg, skip_b2)

    nc.compile()
    return nc


def _emit_body(nc, tc, R, slot, const, work, ps_small, ps_wb, ps_h, ps_out,
               xT_sb, Wg_sb, bg_sb, ones_sb, wTd, ident,
               W1_sb, b1_sb, W2_sb, b2_sb, outT_d, skip_bg=False,
               skip_b2=False):
    # ---- gating: logits token-major [128, tg, e] --------------------------
    L_sb = const.tile([128, TG, E], F32, name=R + "L", tag="L")
    for tg in range(TG):
        psl = ps_small.tile([128, E], F32, name=f"{R}psl{tg}", tag="small")
        for dc in range(DC):
            nc.tensor.matmul(
                psl, xT_sb[dc][:, tg * 128:(tg + 1) * 128], Wg_sb[dc],
                start=(dc == 0), stop=(skip_bg and dc == DC - 1))
        if not skip_bg:
            nc.tensor.matmul(psl, ones_sb, bg_sb, start=False, stop=True)
        nc.scalar.copy(L_sb[:, tg, :], psl)

    # ---- softmax over E (innermost) ---------------------------------------
    mx = const.tile([128, TG], F32, name=R + "mx", tag="mx")
    nc.vector.reduce_max(out=mx, in_=L_sb, axis=mybir.AxisListType.X)
    P8 = const.tile([128, TG, E], F32, name=R + "P8", tag="P8")
    nc.vector.tensor_sub(P8, L_sb, _bcast_inner(mx, E))
    nc.scalar.activation(P8, P8, mybir.ActivationFunctionType.Exp)
    sm = const.tile([128, TG], F32, name=R + "sm", tag="sm")
    nc.vector.reduce_sum(out=sm, in_=P8, axis=mybir.AxisListType.X)
    rs = const.tile([128, TG], F32, name=R + "rs", tag="rs")
    nc.vector.reciprocal(rs, sm)
    nc.vector.tensor_mul(P8, P8, _bcast_inner(rs, E))

    # ---- sort descending: iterative max extraction ------------------------
    # rank-major layout [128, rank, tg]: one PE transpose of the flat [128,
    # 64] view then yields [e*TG+tg, tok] partitions, which a single DMA can
    # lay out as [E, TOK] in DRAM.
    ws = const.tile([128, E, TG], F32, name=R + "ws", tag="ws")
    eq = const.tile([128, TG, E], F32, name=R + "eq", tag="eq")
    for r in range(E):
        nc.vector.reduce_max(out=ws[:, r, :], in_=P8,
                             axis=mybir.AxisListType.X)
        if r < E - 1:
            nc.vector.tensor_tensor(
                eq, P8, _bcast_inner(ws[:, r, :], E),
                op=mybir.AluOpType.is_equal)
            # P8 += -2 * eq  (knock out the extracted max)
            nc.vector.scalar_tensor_tensor(
                P8, eq, -2.0, P8,
                op0=mybir.AluOpType.mult, op1=mybir.AluOpType.add)

    # ---- experts ----------------------------------------------------------
    # Emission order keeps PE streaming: expert 0's first matmuls are emitted
    # before the sorted-weight transposes, so the PE fills the DVE sort
    # latency with useful work. Sorted weights go to bf16 hi+lo pairs so the
    # broadcast / b2 matmuls run at bf16 rate (w = hi + lo, exact to 2^-18).
    h_sb = [[const.tile([128, TOK], BF16, name=f"{R}h_{e}_{hc}",
                        tag=f"h_{e}_{hc}s{slot}")
             for hc in range(HC)] for e in range(E)]
    # [e*TG+tg, 128tok] — partition-order matches DRAM [E, TOK] exactly
    wT_sb = const.tile([E * TG, 128], BF16, name=R + "wT", tag=f"wTs{slot}")

    def emit_ph_pair(e, hc):
        # both token halves accumulate into one 2-bank PSUM tile; single gelu
        ph = ps_h.tile([128, TN, 512], F32, name=f"{R}ph{e}_{hc}", tag="ph")
        for dc in range(DC):
            w_ap = W1_sb[e][dc][:, hc * 128:(hc + 1) * 128]
            for t in range(TN):
                mm = nc.tensor.matmul(ph[:, t, :], w_ap,
                                      xT_sb[dc][:, t * 512:(t + 1) * 512],
                                      start=(dc == 0), stop=(dc == DC - 1))
                if t > 0:
                    mm.ins.ldweights = False
        gt = work.tile([128, TN, 512], BF16, name=f"{R}gt{e}_{hc}",
                       tag="gt", bufs=8)
        nc.scalar.activation(gt, ph, GELU_FUNC, bias=b1_sb[e][hc])
        return gt

    def emit_wb(e, tn):
        # replicate bf16 w row e across 128 partitions via DMA broadcast
        # from the DRAM bounce (stride-0 partition reads need a DRAM source)
        row = wTd[e:e + 1, tn * 512:(tn + 1) * 512]
        bcast = bass.AP(tensor=row.tensor, offset=row.offset,
                        ap=[[0, 128]] + [list(d) for d in row.ap[1:]])
        wb = work.tile([128, 512], BF16, name=f"{R}wb{e}_{tn}", tag="wbs")
        nc.sync.dma_start(wb, bcast)
        return wb

    # experts 0-2: matmuls first (PE cover while DVE runs softmax + sort)
    NPRE = 3
    gt_pre = {e: [emit_ph_pair(e, hc) for hc in range(HC)]
              for e in range(NPRE)}

    # sorted weights -> bf16 -> one PE transpose of the flat [128, 64] view
    # -> [64, 128] psum -> one DVE copy -> one DMA to the DRAM bounce
    ws_bf = const.tile([128, E * TG], BF16, name=R + "wsbf", tag="wsbf")
    nc.vector.tensor_copy(ws_bf, ws)
    pst = ps_out.tile([E * TG, 128], BF16, name=R + "pst", tag="po")
    nc.tensor.transpose(pst, ws_bf, ident)
    nc.vector.tensor_copy(wT_sb, pst)
    wTd_flat = bass.AP(tensor=wTd[:, :].tensor, offset=0,
                       ap=[[128, E * TG], [1, 128]])
    nc.sync.dma_start(wTd_flat, wT_sb)

    for e in range(NPRE):
        wbs = [emit_wb(e, tn) for tn in range(TN)]
        for hc in range(HC):
            for tn in range(TN):
                tsl = slice(tn * 512, (tn + 1) * 512)
                nc.vector.tensor_mul(h_sb[e][hc][:, tsl],
                                     gt_pre[e][hc][:, tn, :], wbs[tn])

    for e in range(NPRE, E):
        wbs = [emit_wb(e, tn) for tn in range(TN)]
        for hc in range(HC):
            gt = emit_ph_pair(e, hc)
            for tn in range(TN):
                tsl = slice(tn * 512, (tn + 1) * 512)
                nc.vector.tensor_mul(h_sb[e][hc][:, tsl], gt[:, tn, :],
                                     wbs[tn])

    # Scheduler fence: keep the second GEMM after all first-GEMM work so the
    # PE never reaches an h-consuming matmul before its producers have cover.
    tc.no_sync_barrier()

    for oc in range(OC):
        pos = [ps_out.tile([128, 512], F32, name=f"{R}po{oc}_{t}", tag="po")
               for t in range(TN)]
        if not skip_b2:
            # bf16 b2 x bf16 wT (rare path: only when b2 is nonzero)
            for t in range(TN):
                mm = nc.tensor.matmul(pos[t], b2_sb[:, oc * 128:(oc + 1) * 128],
                                      wT_sb[:, t * 512:(t + 1) * 512],
                                      start=True, stop=False)
                if t > 0:
                    mm.ins.ldweights = False
        for e in range(E):
            for hc in range(HC):
                w_ap = W2_sb[e][hc][:, oc * 128:(oc + 1) * 128]
                for t in range(TN):
                    mm = nc.tensor.matmul(
                        pos[t], w_ap, h_sb[e][hc][:, t * 512:(t + 1) * 512],
                        start=(skip_b2 and e == 0 and hc == 0),
                        stop=(e == E - 1 and hc == HC - 1))
                    if t > 0:
                        mm.ins.ldweights = False
        for t in range(TN):
            ot = work.tile([128, 512], BF16, name=f"{R}ot{oc}_{t}", tag="ot")
            nc.vector.tensor_copy(ot, pos[t])
            nc.sync.dma_start(
                outT_d[oc * 128:(oc + 1) * 128, t * 512:(t + 1) * 512], ot)


def _prep_in_maps(x, Wg, bg, W1, b1, W2, b2):
    x = np.asarray(x, dtype=np.float32).reshape(B * S, D)
    Wg_bf = np.asarray(Wg, dtype=np.float32).astype(ml_dtypes.bfloat16)
    bg_f = np.asarray(bg, dtype=np.float32).reshape(1, E)
    W1_bf = np.asarray(W1, dtype=np.float32).astype(ml_dtypes.bfloat16)
    b1_f = np.ascontiguousarray(
        np.asarray(b1, dtype=np.float32).reshape(E, HC, 128, 1))
    W2_bf = np.asarray(W2, dtype=np.float32).astype(ml_dtypes.bfloat16)
    b2_f = np.asarray(b2, dtype=np.float32).astype(ml_dtypes.bfloat16)
    in_maps = []
    for c in range(NCORES):
        xc = x[c * TOK:(c + 1) * TOK]                      # [TOK, D]
        xT = np.ascontiguousarray(xc.T).astype(ml_dtypes.bfloat16)
        in_maps.append({
            "xT": xT, "Wg": Wg_bf, "bg": bg_f, "W1": W1_bf,
            "b1": b1_f, "W2": W2_bf, "b2": b2_f,
        })
    return in_maps


def kernel(x, Wg, bg, W1, b1, W2, b2):
    from concourse.bass_utils import run_bass_kernel_spmd

    zbg = not np.any(np.asarray(bg, dtype=np.float32))
    zb2 = not np.any(np.asarray(b2, dtype=np.float32))
    key = ("nc", zbg, zb2)
    if key not in _CACHE:
        _CACHE[key] = build_nc(skip_bg=zbg, skip_b2=zb2)
    nc = _CACHE[key]
    in_maps = _prep_in_maps(x, Wg, bg, W1, b1, W2, b2)
    res = run_bass_kernel_spmd(nc, in_maps, core_ids=list(range(NCORES)))
    out = np.empty((B * S, DOUT), dtype=np.float32)
    for c in range(NCORES):
        out[c * TOK:(c + 1) * TOK] = res.results[c]["outT"].T
    return out.reshape(B, S, DOUT)



# revision 33
# speedup vs baseline: 1.0593x; 1.0257x over previous
"""MoE layer (dense routing, E=8 experts, top_k=E) Trainium2 Bass kernel.

Problem (hardcoded): x [4, 2048, 1024] f32, Wg [1024, 8], bg [8],
W1 [8, 1024, 256], b1 [8, 256], W2 [8, 256, 1024], b2 [8, 1024].

reference:
    logits = x @ Wg + bg ; probs = softmax(logits)
    sorted_probs = sort(probs, descending)          # top_k with k=E
    h_e = gelu(x @ W1[e] + b1[e])                   # all experts, all tokens
    out = sum_e (h_e @ W2[e] + b2[e]) * sorted_probs[..., e, None]

Sharding: data-parallel over the 8192 tokens -> 1024 tokens/core, 8 cores,
weights replicated, no collectives.

Per-core layout: activations are feature-major ([D, tok] / [H, tok] /
[DOUT, tok]); the host pre-transposes x and un-transposes the output.
All big matmuls are bf16 (fp32 PSUM accumulation). Gating softmax + the
descending sort (iterative max-extraction) run token-major on DVE in a
rank-major [128, E, TG] layout; one PE transpose of the flat [128, 64]
view yields [e*TG+tg, tok] partitions, one DVE copy + one DMA lays that
out as bf16 [E, TOK] in DRAM, and per-expert stride-0-partition DMAs
broadcast each row across 128 partitions. gelu(bias) runs on Act over a
paired 2-bank PSUM tile; the per-token weighting mul runs bf16 on DVE.
The second matmul accumulates all 8 experts (and the b2 @ w term) into
one PSUM tile per output chunk.

Loop mode emits 4 bodies per For_i iteration (staggered_reset, h/wT
double-buffered by body parity, a no_sync scheduling fence before each
body's second GEMM) so consecutive bodies pipeline across engines.
test.py uses the marginal time between loop counts to measure HW exec
time through the fixed ~40ms axon dispatch overhead, timing the two
points in alternating pairs so slow device-clock drift cancels.
"""

import sys

if "/opt/trn_rl_repo" not in sys.path:
    sys.path.insert(0, "/opt/trn_rl_repo")

import numpy as np
import ml_dtypes

import concourse.bass as bass
import concourse.mybir as mybir
import concourse.tile as tile
from concourse import bacc
from concourse.masks import make_identity

B, S, D, DOUT = 4, 2048, 1024, 1024
E, H = 8, 256
NCORES = 8
TOK = (B * S) // NCORES  # 1024 tokens per core
DC = D // 128            # 8 contraction chunks over D
HC = H // 128            # 2 chunks over H
OC = DOUT // 128         # 8 chunks over DOUT
TN = TOK // 512          # 2 moving-dim chunks of 512 tokens
TG = TOK // 128          # 8 token groups of 128 (partition tiles)

BF16 = mybir.dt.bfloat16
F32 = mybir.dt.float32

_CACHE = {}
GELU_FUNC = mybir.ActivationFunctionType.Gelu  # sim override hook


def _bcast_inner(ap2d, n):
    """[P, G] (or [P, G, 1]) AP -> [P, G, n] AP with stride-0 innermost."""
    a = [list(d) for d in ap2d.ap]
    if len(a) == 3:
        assert a[2][1] == 1
        a = a[:2]
    return bass.AP(tensor=ap2d.tensor, offset=ap2d.offset, ap=a + [[0, n]])


def build_nc(reps=1, loop_n=None, skip_bg=False, skip_b2=False):
    nc = bacc.Bacc("TRN2", target_bir_lowering=False, debug=False,
                   num_devices=NCORES)

    xT_d = nc.dram_tensor("xT", [D, TOK], BF16, kind="ExternalInput")
    Wg_d = nc.dram_tensor("Wg", [D, E], BF16, kind="ExternalInput")
    bg_d = nc.dram_tensor("bg", [1, E], F32, kind="ExternalInput")
    W1_d = nc.dram_tensor("W1", [E, D, H], BF16, kind="ExternalInput")
    b1_d = nc.dram_tensor("b1", [E, HC, 128, 1], F32, kind="ExternalInput")
    W2_d = nc.dram_tensor("W2", [E, H, DOUT], BF16, kind="ExternalInput")
    b2_d = nc.dram_tensor("b2", [E, DOUT], BF16, kind="ExternalInput")
    outT_d = nc.dram_tensor("outT", [DOUT, TOK], BF16, kind="ExternalOutput")
    wTd2 = [nc.dram_tensor(f"wT_scratch{s}", [E, TOK], BF16) for s in range(2)]

    with tile.TileContext(nc) as tc:
        with (
            tc.tile_pool(name="const", bufs=1) as const,
            tc.tile_pool(name="work", bufs=4) as work,
            tc.tile_pool(name="ps_small", bufs=1, space="PSUM") as ps_small,
            tc.tile_pool(name="ps_h", bufs=2, space="PSUM") as ps_h,
            tc.tile_pool(name="ps_out", bufs=3, space="PSUM") as ps_out,
        ):
            # ---- resident inputs ------------------------------------------
            xT_sb = []
            for dc in range(DC):
                t = const.tile([128, TOK], BF16, name=f"xT{dc}")
                nc.sync.dma_start(t, xT_d[dc * 128:(dc + 1) * 128, :])
                xT_sb.append(t)
            Wg_sb = []
            for dc in range(DC):
                t = const.tile([128, E], BF16, name=f"Wg{dc}")
                nc.sync.dma_start(t, Wg_d[dc * 128:(dc + 1) * 128, :])
                Wg_sb.append(t)
            bg_sb = const.tile([1, E], F32, name="bg")
            nc.sync.dma_start(bg_sb, bg_d[:, :])
            ones_sb = const.tile([1, 128], F32, name="ones")
            nc.vector.memset(ones_sb, 1.0)
            ident = const.tile([128, 128], F32, name="ident")
            make_identity(nc, ident)
            ident_bf = const.tile([128, 128], BF16, name="ident_bf")
            nc.vector.tensor_copy(ident_bf, ident)

            W1_sb = [[None] * DC for _ in range(E)]
            b1_sb = [[None] * HC for _ in range(E)]
            for e in range(E):
                for dc in range(DC):
                    t = const.tile([128, H], BF16, name=f"W1_{e}_{dc}")
                    nc.sync.dma_start(t, W1_d[e, dc * 128:(dc + 1) * 128, :])
                    W1_sb[e][dc] = t
                for hc in range(HC):
                    t = const.tile([128, 1], F32, name=f"b1_{e}_{hc}")
                    nc.sync.dma_start(t, b1_d[e, hc, :, :])
                    b1_sb[e][hc] = t
            W2_sb = [[None] * HC for _ in range(E)]
            for e in range(E):
                for hc in range(HC):
                    t = const.tile([128, DOUT], BF16, name=f"W2_{e}_{hc}")
                    nc.sync.dma_start(t, W2_d[e, hc * 128:(hc + 1) * 128, :])
                    W2_sb[e][hc] = t
            b2_sb = const.tile([E, DOUT], BF16, name="b2")
            nc.sync.dma_start(b2_sb, b2_d[:, :])

            if loop_n is not None:
                n4, rem = divmod(loop_n, 4)
                if n4 > 0:
                    with tc.For_i(0, n4, 1,
                                  hint_engines=(mybir.EngineType.PE,),
                                  staggered_reset=True):
                        for s in range(4):
                            _emit_body(nc, tc, f"lp{s}_", s % 2, const, work,
                                       ps_small, ps_small, ps_h, ps_out,
                                       xT_sb, Wg_sb, bg_sb, ones_sb,
                                       wTd2[s % 2], ident_bf, W1_sb, b1_sb,
                                       W2_sb, b2_sb, outT_d, skip_bg,
                                       skip_b2)
                for r in range(rem):
                    _emit_body(nc, tc, f"lr{r}_", r % 2, const, work,
                               ps_small, ps_small, ps_h, ps_out, xT_sb,
                               Wg_sb, bg_sb, ones_sb, wTd2[r % 2], ident_bf,
                               W1_sb, b1_sb, W2_sb, b2_sb, outT_d,
                               skip_bg, skip_b2)
            else:
                for rep in range(reps):
                    _emit_body(nc, tc, f"r{rep}_", rep % 2, const, work,
                               ps_small, ps_small, ps_h, ps_out, xT_sb,
                               Wg_sb, bg_sb, ones_sb, wTd2[rep % 2], ident_bf,
                               W1_sb, b1_sb, W2_sb, b2_sb, outT_d,
                               skip_bg, skip_b2)

    nc.compile()
    return nc


def _emit_body(nc, tc, R, slot, const, work, ps_small, ps_wb, ps_h, ps_out,
               xT_sb, Wg_sb, bg_sb, ones_sb, wTd, ident,
               W1_sb, b1_sb, W2_sb, b2_sb, outT_d, skip_bg=False,
               skip_b2=False):
    # ---- gating: logits token-major [128, tg, e] --------------------------
    L_sb = const.tile([128, TG, E], F32, name=R + "L", tag="L")
    for tg in range(TG):
        psl = ps_small.tile([128, E], F32, name=f"{R}psl{tg}", tag="small")
        for dc in range(DC):
            nc.tensor.matmul(
                psl, xT_sb[dc][:, tg * 128:(tg + 1) * 128], Wg_sb[dc],
                start=(dc == 0), stop=(skip_bg and dc == DC - 1))
        if not skip_bg:
            nc.tensor.matmul(psl, ones_sb, bg_sb, start=False, stop=True)
        nc.scalar.copy(L_sb[:, tg, :], psl)

    # ---- softmax over E (innermost) ---------------------------------------
    mx = const.tile([128, TG], F32, name=R + "mx", tag="mx")
    nc.vector.reduce_max(out=mx, in_=L_sb, axis=mybir.AxisListType.X)
    P8 = const.tile([128, TG, E], F32, name=R + "P8", tag="P8")
    nc.vector.tensor_sub(P8, L_sb, _bcast_inner(mx, E))
    nc.scalar.activation(P8, P8, mybir.ActivationFunctionType.Exp)
    sm = const.tile([128, TG], F32, name=R + "sm", tag="sm")
    nc.vector.reduce_sum(out=sm, in_=P8, axis=mybir.AxisListType.X)
    rs = const.tile([128, TG], F32, name=R + "rs", tag="rs")
    nc.vector.reciprocal(rs, sm)
    nc.vector.tensor_mul(P8, P8, _bcast_inner(rs, E))

    # ---- sort descending: iterative max extraction ------------------------
    # rank-major layout [128, rank, tg]: one PE transpose of the flat [128,
    # 64] view then yields [e*TG+tg, tok] partitions, which a single DMA can
    # lay out as [E, TOK] in DRAM.
    ws = const.tile([128, E, TG], F32, name=R + "ws", tag="ws")
    eq = const.tile([128, TG, E], F32, name=R + "eq", tag="eq")
    for r in range(E):
        nc.vector.reduce_max(out=ws[:, r, :], in_=P8,
                             axis=mybir.AxisListType.X)
        if r < E - 1:
            nc.vector.tensor_tensor(
                eq, P8, _bcast_inner(ws[:, r, :], E),
                op=mybir.AluOpType.is_equal)
            # P8 += -2 * eq  (knock out the extracted max)
            nc.vector.scalar_tensor_tensor(
                P8, eq, -2.0, P8,
                op0=mybir.AluOpType.mult, op1=mybir.AluOpType.add)

    # ---- experts ----------------------------------------------------------
    # Emission order keeps PE streaming: expert 0's first matmuls are emitted
    # before the sorted-weight transposes, so the PE fills the DVE sort
    # latency with useful work. Sorted weights go to bf16 hi+lo pairs so the
    # broadcast / b2 matmuls run at bf16 rate (w = hi + lo, exact to 2^-18).
    h_sb = [[const.tile([128, TOK], BF16, name=f"{R}h_{e}_{hc}",
                        tag=f"h_{e}_{hc}s{slot}")
             for hc in range(HC)] for e in range(E)]
    # [e*TG+tg, 128tok] — partition-order matches DRAM [E, TOK] exactly
    wT_sb = const.tile([E * TG, 128], BF16, name=R + "wT", tag=f"wTs{slot}")

    def emit_ph_pair(e, hc):
        # both token halves accumulate into one 2-bank PSUM tile; single gelu
        ph = ps_h.tile([128, TN, 512], F32, name=f"{R}ph{e}_{hc}", tag="ph")
        for dc in range(DC):
            w_ap = W1_sb[e][dc][:, hc * 128:(hc + 1) * 128]
            for t in range(TN):
                mm = nc.tensor.matmul(ph[:, t, :], w_ap,
                                      xT_sb[dc][:, t * 512:(t + 1) * 512],
                                      start=(dc == 0), stop=(dc == DC - 1))
                if t > 0:
                    mm.ins.ldweights = False
        gt = work.tile([128, TN, 512], BF16, name=f"{R}gt{e}_{hc}",
                       tag="gt", bufs=8)
        nc.scalar.activation(gt, ph, GELU_FUNC, bias=b1_sb[e][hc])
        return gt

    def emit_wb(e, tn):
        # replicate bf16 w row e across 128 partitions via DMA broadcast
        # from the DRAM bounce (stride-0 partition reads need a DRAM source)
        row = wTd[e:e + 1, tn * 512:(tn + 1) * 512]
        bcast = bass.AP(tensor=row.tensor, offset=row.offset,
                        ap=[[0, 128]] + [list(d) for d in row.ap[1:]])
        wb = work.tile([128, 512], BF16, name=f"{R}wb{e}_{tn}", tag="wbs")
        nc.sync.dma_start(wb, bcast)
        return wb

    # experts 0-2: matmuls first (PE cover while DVE runs softmax + sort)
    NPRE = 3
    gt_pre = {e: [emit_ph_pair(e, hc) for hc in range(HC)]
              for e in range(NPRE)}

    # sorted weights -> bf16 -> one PE transpose of the flat [128, 64] view
    # -> [64, 128] psum -> one DVE copy -> one DMA to the DRAM bounce
    ws_bf = const.tile([128, E * TG], BF16, name=R + "wsbf", tag="wsbf")
    nc.vector.tensor_copy(ws_bf, ws)
    pst = ps_out.tile([E * TG, 128], BF16, name=R + "pst", tag="po")
    nc.tensor.transpose(pst, ws_bf, ident)
    nc.vector.tensor_copy(wT_sb, pst)
    wTd_flat = bass.AP(tensor=wTd[:, :].tensor, offset=0,
                       ap=[[128, E * TG], [1, 128]])
    nc.sync.dma_start(wTd_flat, wT_sb)
    wTE = None
    if not skip_b2:
        # [E, TOK] copy of the sorted weights for the b2 @ w matmuls
        wTE = const.tile([E, TOK], BF16, name=R + "wTE", tag=f"wTE{slot}")
        nc.sync.dma_start(wTE, wTd[:, :])

    for e in range(NPRE):
        wbs = [emit_wb(e, tn) for tn in range(TN)]
        for hc in range(HC):
            for tn in range(TN):
                tsl = slice(tn * 512, (tn + 1) * 512)
                nc.vector.tensor_mul(h_sb[e][hc][:, tsl],
                                     gt_pre[e][hc][:, tn, :], wbs[tn])

    for e in range(NPRE, E):
        wbs = [emit_wb(e, tn) for tn in range(TN)]
        for hc in range(HC):
            gt = emit_ph_pair(e, hc)
            for tn in range(TN):
                tsl = slice(tn * 512, (tn + 1) * 512)
                nc.vector.tensor_mul(h_sb[e][hc][:, tsl], gt[:, tn, :],
                                     wbs[tn])

    # Scheduler fence: keep the second GEMM after all first-GEMM work so the
    # PE never reaches an h-consuming matmul before its producers have cover.
    tc.no_sync_barrier()

    for oc in range(OC):
        pos = [ps_out.tile([128, 512], F32, name=f"{R}po{oc}_{t}", tag="po")
               for t in range(TN)]
        if not skip_b2:
            # bf16 b2 x bf16 wT (rare path: only when b2 is nonzero)
            for t in range(TN):
                mm = nc.tensor.matmul(pos[t], b2_sb[:, oc * 128:(oc + 1) * 128],
                                      wTE[:, t * 512:(t + 1) * 512],
                                      start=True, stop=False)
                if t > 0:
                    mm.ins.ldweights = False
        for e in range(E):
            for hc in range(HC):
                w_ap = W2_sb[e][hc][:, oc * 128:(oc + 1) * 128]
                for t in range(TN):
                    mm = nc.tensor.matmul(
                        pos[t], w_ap, h_sb[e][hc][:, t * 512:(t + 1) * 512],
                        start=(skip_b2 and e == 0 and hc == 0),
                        stop=(e == E - 1 and hc == HC - 1))
                    if t > 0:
                        mm.ins.ldweights = False
        for t in range(TN):
            ot = work.tile([128, 512], BF16, name=f"{R}ot{oc}_{t}", tag="ot")
            nc.vector.tensor_copy(ot, pos[t])
            nc.sync.dma_start(
                outT_d[oc * 128:(oc + 1) * 128, t * 512:(t + 1) * 512], ot)


def _prep_in_maps(x, Wg, bg, W1, b1, W2, b2):
    x = np.asarray(x, dtype=np.float32).reshape(B * S, D)
    Wg_bf = np.asarray(Wg, dtype=np.float32).astype(ml_dtypes.bfloat16)
    bg_f = np.asarray(bg, dtype=np.float32).reshape(1, E)
    W1_bf = np.asarray(W1, dtype=np.float32).astype(ml_dtypes.bfloat16)
    b1_f = np.ascontiguousarray(
        np.asarray(b1, dtype=np.float32).reshape(E, HC, 128, 1))
    W2_bf = np.asarray(W2, dtype=np.float32).astype(ml_dtypes.bfloat16)
    b2_f = np.asarray(b2, dtype=np.float32).astype(ml_dtypes.bfloat16)
    in_maps = []
    for c in range(NCORES):
        xc = x[c * TOK:(c + 1) * TOK]                      # [TOK, D]
        xT = np.ascontiguousarray(xc.T).astype(ml_dtypes.bfloat16)
        in_maps.append({
            "xT": xT, "Wg": Wg_bf, "bg": bg_f, "W1": W1_bf,
            "b1": b1_f, "W2": W2_bf, "b2": b2_f,
        })
    return in_maps


def kernel(x, Wg, bg, W1, b1, W2, b2):
    from concourse.bass_utils import run_bass_kernel_spmd

    zbg = not np.any(np.asarray(bg, dtype=np.float32))
    zb2 = not np.any(np.asarray(b2, dtype=np.float32))
    key = ("nc", zbg, zb2)
    if key not in _CACHE:
        _CACHE[key] = build_nc(skip_bg=zbg, skip_b2=zb2)
    nc = _CACHE[key]
    in_maps = _prep_in_maps(x, Wg, bg, W1, b1, W2, b2)
    res = run_bass_kernel_spmd(nc, in_maps, core_ids=list(range(NCORES)))
    out = np.empty((B * S, DOUT), dtype=np.float32)
    for c in range(NCORES):
        out[c * TOK:(c + 1) * TOK] = res.results[c]["outT"].T
    return out.reshape(B, S, DOUT)

